# revision 8
# baseline (speedup 1.0000x reference)
"""Additive (Bahdanau) attention on 8 TRN2 NeuronCores.

Reference computation:
    qp = queries @ W_q                  (bs, n_q, 64)
    kp = keys @ W_k                     (bs, n_k, 64)
    scores[b,q,k] = sum_h w_v[h] * tanh(qp[b,q,h] + kp[b,k,h])
    out = softmax(scores, -1) @ values

Key trick: tanh(x) on [-9.9, 9.9] is approximated by a sum of J=7 ODD
harmonics of one base frequency (max err 5.4e-3):
    tanh(x) ~= sum_j c_j sin((2j+1) w0 x)
Angle addition makes the score computation separable:
    sin(w(a+b)) = sin(wa)cos(wb) + cos(wa)sin(wb)
so scores reduce to matmuls with contraction dim J*2*64 — pure TensorEngine
work. The giant (bs, n_q, n_k, 64) tanh tensor of the naive implementation
never exists.

The harmonics sin/cos((2j+1)theta) are built WITHOUT trig activations via
the Chebyshev 3-term recurrence (valid for both sin and cos rows):
    P_{k+2} = 2cos(2theta) * P_k - P_{k-2}
seeded by P_1 = [.|.](+theta) and P_{-1} = [.|.](-theta) from two Sin
activations (|theta| <= 1.3 so no range reduction), with
    2cos(2theta) = [sin rows] 2-4sin^2  /  [cos rows] 4cos^2-2
computed from P_1 by one square + one per-partition affine. Everything is
fp32; drift over 6 steps is ~1e-6.

Scores are built TRANSPOSED (k on partitions, q free) so the attention
weights feed the output matmul with no transposes:
    outT (v, q) = sum_kt values[kt] (lhsT) @ expT[kt]
    sums (1, q) = sum_kt ones^T @ expT[kt]
and only the final (v, q) -> (q, v) transpose + per-partition normalize
remain.

Sharding: fully data-parallel, no collectives. Core c handles batch c//2,
query half c%2: (512 q, 1024 k).
"""

import numpy as np

BS, NQ, NK = 4, 1024, 1024
QD, KD, VD, HID = 128, 128, 128, 64
NCORES = 8
NQH = NQ // 2  # queries per core

J = 7
W0 = 0.249227
FOURIER_C = [1.2417762, 0.3409618, 0.1432009, 0.064270845,
             0.029813108, 0.013750755, 0.0055594866]

HALF_PI = 1.5707963267948966

_CACHED = {}


def _build():
    import concourse.bacc as bacc
    import concourse.mybir as mybir
    from concourse import tile
    from concourse.alu_op_type import AluOpType
    from concourse.masks import make_identity

    F32 = mybir.dt.float32
    BF16 = mybir.dt.bfloat16
    A = mybir.ActivationFunctionType

    nc = bacc.Bacc(None, target_bir_lowering=False)

    q_sh = nc.declare_dram_parameter("q_sh", [NQH, QD], F32, isOutput=False)
    k_sh = nc.declare_dram_parameter("k_sh", [NK, KD], F32, isOutput=False)
    v_sh = nc.declare_dram_parameter("v_sh", [NK, VD], F32, isOutput=False)
    wq2 = nc.declare_dram_parameter("wq2", [QD, 128], F32, isOutput=False)
    wk2 = nc.declare_dram_parameter("wk2", [KD, 128], F32, isOutput=False)
    cw = nc.declare_dram_parameter("cw", [128, J], F32, isOutput=False)
    biasq = nc.declare_dram_parameter("biasq", [128, 1], F32, isOutput=False)
    biask = nc.declare_dram_parameter("biask", [128, 1], F32, isOutput=False)
    c2c = nc.declare_dram_parameter("c2c", [128, 4], F32, isOutput=False)
    out = nc.declare_dram_parameter("out", [NQH, VD], F32, isOutput=True)

    NQC = NQH // 128  # 4 query chunks
    NKC = NK // 128   # 8 key chunks

    with tile.TileContext(nc) as tc:
        with (
            tc.tile_pool(name="consts", bufs=1) as consts,
            tc.tile_pool(name="io", bufs=1) as io,
            tc.tile_pool(name="chunks", bufs=4) as chunks,
            tc.tile_pool(name="vals", bufs=NKC) as vals,
            tc.tile_pool(name="kch", bufs=4) as kch,
            tc.tile_pool(name="qch", bufs=4) as qch,
            tc.tile_pool(name="jbank", bufs=3) as jbank,
            tc.tile_pool(name="sm", bufs=NKC) as sm,
            tc.tile_pool(name="ps", bufs=8, space="PSUM") as ps,
        ):
            # ---- constants ----
            id32 = consts.tile([128, 128], F32, tag="id32")
            make_identity(nc, id32[:])
            ones16 = consts.tile([128, 1], BF16, tag="ones16")
            nc.gpsimd.memset(ones16[:], 1.0)
            wq2_sb = consts.tile([QD, 128], F32, tag="wq2")
            wk2_sb = consts.tile([KD, 128], F32, tag="wk2")
            cw_sb = consts.tile([128, J], F32, tag="cw")
            biasq_sb = consts.tile([128, 1], F32, tag="biasq")
            biask_sb = consts.tile([128, 1], F32, tag="biask")
            c2c_sb = consts.tile([128, 4], F32, tag="c2c")
            nc.sync.dma_start(wq2_sb[:], wq2[:])
            nc.sync.dma_start(wk2_sb[:], wk2[:])
            nc.sync.dma_start(cw_sb[:], cw[:])
            nc.sync.dma_start(biasq_sb[:], biasq[:])
            nc.sync.dma_start(biask_sb[:], biask[:])
            nc.sync.dma_start(c2c_sb[:], c2c[:])

            # ---- inputs: q/k spread across queues, transpose to (d, n) ----
            qT = io.tile([QD, NQH], F32, tag="qT")
            kT = io.tile([KD, NK], F32, tag="kT")
            for i in range(NQC):
                qc = chunks.tile([128, QD], F32, tag="qc")
                nc.sync.dma_start(qc[:], q_sh[i * 128:(i + 1) * 128, :])
                p = ps.tile([128, 512], F32, tag="t512")
                nc.tensor.transpose(p[:, :128], qc[:], id32[:])
                nc.scalar.copy(qT[:, i * 128:(i + 1) * 128], p[:, :128])
            for i in range(NKC):
                kc_t = chunks.tile([128, KD], F32, tag="kc")
                nc.scalar.dma_start(kc_t[:], k_sh[i * 128:(i + 1) * 128, :])
                p = ps.tile([128, 512], F32, tag="t512")
                nc.tensor.transpose(p[:, :128], kc_t[:], id32[:])
                nc.scalar.copy(kT[:, i * 128:(i + 1) * 128], p[:, :128])
            # values: needed only at the tail; own queue, cast to bf16
            v16 = []
            for i in range(NKC):
                vc = chunks.tile([128, VD], F32, tag="vc")
                nc.gpsimd.dma_start(vc[:], v_sh[i * 128:(i + 1) * 128, :])
                vb = vals.tile([128, VD], BF16, tag="v16", name=f"v16_{i}")
                nc.vector.tensor_copy(vb[:], vc[:])
                v16.append(vb)

            # ---- projections: packed (2x64 h, n) = [W | W]^T @ xT ----
            qp2 = io.tile([128, NQH], F32, tag="qp2")
            kp2 = io.tile([128, NK], F32, tag="kp2")
            p = ps.tile([128, 512], F32, tag="t512")
            nc.tensor.matmul(p[:], wq2_sb[:], qT[:], start=True, stop=True)
            nc.scalar.copy(qp2[:], p[:])
            for c in range(2):
                p = ps.tile([128, 512], F32, tag="t512")
                nc.tensor.matmul(p[:], wk2_sb[:], kT[:, c * 512:(c + 1) * 512],
                                 start=True, stop=True)
                nc.scalar.copy(kp2[:, c * 512:(c + 1) * 512], p[:])

            # ---- base harmonics (theta = w0*x; K rows [cos|sin], Q [sin|cos]) --
            p1k = kch.tile([128, NK], F32, tag="kchain", name="p1k")
            pm1k = kch.tile([128, NK], F32, tag="kchain", name="pm1k")
            p1q = qch.tile([128, NQH], F32, tag="qchain", name="p1q")
            pm1q = qch.tile([128, NQH], F32, tag="qchain", name="pm1q")
            nc.scalar.activation(p1k[:], kp2[:], A.Sin, bias=biask_sb[:], scale=W0)
            nc.scalar.activation(p1q[:], qp2[:], A.Sin, bias=biasq_sb[:], scale=W0)
            nc.scalar.activation(pm1k[:], kp2[:], A.Sin, bias=biask_sb[:], scale=-W0)
            nc.scalar.activation(pm1q[:], qp2[:], A.Sin, bias=biasq_sb[:], scale=-W0)
            # 2cos(2theta) via square + per-partition affine
            t2k = kch.tile([128, NK], F32, tag="ktmp", name="t2k")
            c2k = io.tile([128, NK], F32, tag="c2k")
            nc.vector.tensor_mul(t2k[:], p1k[:], p1k[:])
            nc.vector.tensor_scalar(c2k[:], t2k[:], c2c_sb[:, 0:1], c2c_sb[:, 1:2],
                                    AluOpType.mult, AluOpType.add)
            t2q = qch.tile([128, NQH], F32, tag="qtmp", name="t2q")
            c2q = io.tile([128, NQH], F32, tag="c2q")
            nc.gpsimd.tensor_mul(t2q[:], p1q[:], p1q[:])
            nc.vector.tensor_scalar(c2q[:], t2q[:], c2c_sb[:, 2:3], c2c_sb[:, 3:4],
                                    AluOpType.mult, AluOpType.add)

            # ---- per-j banks + transposed score accumulation over j ----
            psT = [ps.tile([128, 512], F32, tag="t512", name=f"psT_{kt}")
                   for kt in range(NKC)]

            def score_mms(ks, sq, first, last):
                for kt in range(NKC):
                    nc.tensor.matmul(psT[kt][:],
                                     ks[:, kt * 128:(kt + 1) * 128], sq[:],
                                     start=first, stop=last)

            # j = 0 banks
            ks0 = jbank.tile([128, NK], BF16, tag="ks", name="ks_0")
            nc.vector.tensor_copy(ks0[:], p1k[:])
            sq0 = jbank.tile([128, NQH], BF16, tag="sq", name="sq_0")
            nc.scalar.mul(sq0[:], p1q[:], cw_sb[:, 0:1])
            score_mms(ks0, sq0, first=True, last=False)

            kprev2, kprev = pm1k, p1k
            qprev2, qprev = pm1q, p1q
            for j in range(1, J):
                # K side chain on DVE
                tk = kch.tile([128, NK], F32, tag="ktmp", name=f"tk_{j}")
                pk = kch.tile([128, NK], F32, tag="kchain", name=f"pk_{j}")
                nc.vector.tensor_mul(tk[:], c2k[:], kprev[:])
                nc.vector.tensor_sub(pk[:], tk[:], kprev2[:])
                ksj = jbank.tile([128, NK], BF16, tag="ks", name=f"ks_{j}")
                nc.vector.tensor_copy(ksj[:], pk[:])
                # Q side chain on GpSimd
                tq = qch.tile([128, NQH], F32, tag="qtmp", name=f"tq_{j}")
                pq = qch.tile([128, NQH], F32, tag="qchain", name=f"pq_{j}")
                nc.gpsimd.tensor_mul(tq[:], c2q[:], qprev[:])
                nc.gpsimd.tensor_sub(pq[:], tq[:], qprev2[:])
                sqj = jbank.tile([128, NQH], BF16, tag="sq", name=f"sq_{j}")
                nc.scalar.mul(sqj[:], pq[:], cw_sb[:, j:j + 1])
                score_mms(ksj, sqj, first=False, last=(j == J - 1))
                kprev2, kprev = kprev, pk
                qprev2, qprev = qprev, pq

            # ---- exp (k-major) + denominators via ones-matmul ----
            expT = []
            for kt in range(NKC):
                et = sm.tile([128, 512], BF16, tag="expT", name=f"expT_{kt}")
                nc.scalar.activation(et[:], psT[kt][:], A.Exp)
                expT.append(et)
            psum_sums = ps.tile([1, 512], F32, tag="t512", name="psum_sums")
            for kt in range(NKC):
                nc.tensor.matmul(psum_sums[:], ones16[:], expT[kt][:],
                                 start=(kt == 0), stop=(kt == NKC - 1))
            sums_sb = sm.tile([1, 512], F32, tag="sums_sb")
            nc.scalar.copy(sums_sb[:], psum_sums[:])

            # ---- outT (v, q) = sum_kt values[kt] (as lhsT) @ expT[kt] ----
            ps_outT = ps.tile([128, 512], F32, tag="t512", name="ps_outT")
            for kt in range(NKC):
                nc.tensor.matmul(ps_outT[:], v16[kt][:], expT[kt][:],
                                 start=(kt == 0), stop=(kt == NKC - 1))
            outT_sb = sm.tile([128, 512], F32, tag="outT_sb")
            nc.vector.tensor_copy(outT_sb[:], ps_outT[:])

            # ---- transpose back to (q, v), normalize, store ----
            for qt in range(NQC):
                pcol = ps.tile([128, 512], F32, tag="t512", name=f"pcol_{qt}")
                # (1,128) row -> (128,1) column via 1-deep matmul against [[1.0]]
                nc.tensor.matmul(pcol[:128, :1],
                                 sums_sb[:1, qt * 128:(qt + 1) * 128],
                                 id32[:1, :1], start=True, stop=True)
                rcol = sm.tile([128, 1], F32, tag="rcol", name=f"rcol_{qt}")
                nc.vector.reciprocal(rcol[:], pcol[:128, :1])
                po = ps.tile([128, 512], F32, tag="t512", name=f"po_{qt}")
                nc.tensor.transpose(po[:, :128],
                                    outT_sb[:, qt * 128:(qt + 1) * 128], id32[:])
                o_sb = sm.tile([128, VD], F32, tag="osb", name=f"osb_{qt}")
                nc.vector.tensor_scalar_mul(o_sb[:], po[:, :128], rcol[:])
                nc.sync.dma_start(out[qt * 128:(qt + 1) * 128, :], o_sb[:])

    nc.finalize()
    return nc


def _get_nc():
    if "nc" not in _CACHED:
        _CACHED["nc"] = _build()
    return _CACHED["nc"]


def _make_consts(W_q, W_k, w_v):
    wq2 = np.concatenate([W_q, W_q], axis=1).astype(np.float32)
    wk2 = np.concatenate([W_k, W_k], axis=1).astype(np.float32)
    cw = np.zeros((128, J), np.float32)
    for j in range(J):
        cwj = (FOURIER_C[j] * w_v).astype(np.float32)
        cw[:64, j] = cwj
        cw[64:, j] = cwj
    # Q packing [sin | cos], K packing [cos | sin]
    biasq = np.zeros((128, 1), np.float32)
    biasq[64:] = HALF_PI
    biask = np.full((128, 1), HALF_PI, np.float32)
    biask[64:] = 0.0
    # 2cos(2theta) affine: from sin^2: -4x+2 ; from cos^2: 4x-2
    c2c = np.zeros((128, 4), np.float32)
    c2c[:64, 0], c2c[:64, 1] = 4.0, -2.0    # K rows<64 = cos
    c2c[64:, 0], c2c[64:, 1] = -4.0, 2.0    # K rows>=64 = sin
    c2c[:64, 2], c2c[:64, 3] = -4.0, 2.0    # Q rows<64 = sin
    c2c[64:, 2], c2c[64:, 3] = 4.0, -2.0    # Q rows>=64 = cos
    return wq2, wk2, cw, biasq, biask, c2c


def kernel(queries, keys, values, W_q, W_k, w_v, _trace=False, _trace_kwargs=None):
    from concourse.bass_utils import run_bass_kernel_spmd

    nc = _get_nc()
    wq2, wk2, cw, biasq, biask, c2c = _make_consts(
        np.asarray(W_q), np.asarray(W_k), np.asarray(w_v))
    queries = np.ascontiguousarray(queries, np.float32)
    keys = np.ascontiguousarray(keys, np.float32)
    values = np.ascontiguousarray(values, np.float32)

    in_maps = []
    for c in range(NCORES):
        b, qh = c // 2, c % 2
        in_maps.append({
            "q_sh": np.ascontiguousarray(queries[b, qh * NQH:(qh + 1) * NQH, :]),
            "k_sh": keys[b],
            "v_sh": values[b],
            "wq2": wq2, "wk2": wk2, "cw": cw,
            "biasq": biasq, "biask": biask, "c2c": c2c,
        })

    kwargs = {}
    if _trace:
        kwargs["trace"] = True
        kwargs.update(_trace_kwargs or {})
    res = run_bass_kernel_spmd(nc, in_maps, core_ids=list(range(NCORES)), **kwargs)

    out = np.empty((BS, NQ, VD), np.float32)
    for c in range(NCORES):
        b, qh = c // 2, c % 2
        out[b, qh * NQH:(qh + 1) * NQH, :] = res.results[c]["out"]
    if _trace:
        return out, res
    return out


# revision 9
# speedup vs baseline: 1.3554x; 1.3554x over previous
"""Additive (Bahdanau) attention on 8 TRN2 NeuronCores.

Reference computation:
    qp = queries @ W_q                  (bs, n_q, 64)
    kp = keys @ W_k                     (bs, n_k, 64)
    scores[b,q,k] = sum_h w_v[h] * tanh(qp[b,q,h] + kp[b,k,h])
    out = softmax(scores, -1) @ values

Key trick: tanh(x) on [-9.9, 9.9] is approximated by a sum of J=7 sines
(odd harmonics of w0, max err 5.4e-3):
    tanh(x) ~= sum_j c_j sin(w_j x),  w_j = (2j+1) w0
Angle addition makes the score computation separable:
    sin(w(a+b)) = sin(wa)cos(wb) + cos(wa)sin(wb)
so scores reduce to matmuls with contraction 2*64 per harmonic — pure
TensorEngine work. The giant (bs, n_q, n_k, 64) tanh tensor of the naive
implementation never exists.

sin/cos args reach |w_j x| ~ 18 rad but the ScalarE Sin spline only covers
[-4, 4], so angles are range-reduced exactly in fp32 (j >= 1):
    z = x * (w_j / 2pi) + (S + 32)      # S = 0 (sin half) / 0.25 (cos half)
    r = (z + 2^23) - 2^23               # round-to-nearest via fp32 magic
    g = r - z                           # in [-0.5, 0.5], g = -frac
    sin(w_j x + 2pi S) = Sin(-2pi * g)  # exact periodicity
All pre-steps are exact in fp32; j = 0 feeds Sin directly. Elementwise work
stays on the DVE only (tensor_scalar runs at 2x; GpSimd shares SBUF ports
with the DVE, so offloading there just halves both).

Scores are built TRANSPOSED (k on partitions, q free) so the attention
weights feed the output matmul with no transposes:
    outT (v, q) = sum_kt values[kt] (lhsT) @ expT[kt]
    sums (1, q) = sum_kt ones^T @ expT[kt]
and only a final (v, q) -> (q, v) transpose + per-partition normalize
remain.

Sharding: fully data-parallel, no collectives. Core c handles batch c//2,
query half c%2: (512 q, 1024 k).
"""

import numpy as np

BS, NQ, NK = 4, 1024, 1024
QD, KD, VD, HID = 128, 128, 128, 64
NCORES = 8
NQH = NQ // 2  # queries per core

J = 7
W0 = 0.249227
FOURIER_W = [(2 * j + 1) * W0 for j in range(J)]
FOURIER_C = [1.2417762, 0.3409618, 0.1432009, 0.064270845,
             0.029813108, 0.013750755, 0.0055594866]

MAGIC = 8388608.0  # 2^23
TWO_PI = 6.283185307179586
HALF_PI = 1.5707963267948966

_CACHED = {}


def _build():
    import concourse.bacc as bacc
    import concourse.mybir as mybir
    from concourse import tile
    from concourse.alu_op_type import AluOpType
    from concourse.masks import make_identity

    F32 = mybir.dt.float32
    BF16 = mybir.dt.bfloat16
    A = mybir.ActivationFunctionType

    nc = bacc.Bacc(None, target_bir_lowering=False)

    q_sh = nc.declare_dram_parameter("q_sh", [NQH, QD], F32, isOutput=False)
    k_sh = nc.declare_dram_parameter("k_sh", [NK, KD], F32, isOutput=False)
    v_sh = nc.declare_dram_parameter("v_sh", [NK, VD], F32, isOutput=False)
    wq2 = nc.declare_dram_parameter("wq2", [QD, 128], F32, isOutput=False)
    wk2 = nc.declare_dram_parameter("wk2", [KD, 128], F32, isOutput=False)
    cw = nc.declare_dram_parameter("cw", [128, J], F32, isOutput=False)
    sphq = nc.declare_dram_parameter("sphq", [128, 1], F32, isOutput=False)
    sphk = nc.declare_dram_parameter("sphk", [128, 1], F32, isOutput=False)
    biasq = nc.declare_dram_parameter("biasq", [128, 1], F32, isOutput=False)
    biask = nc.declare_dram_parameter("biask", [128, 1], F32, isOutput=False)
    out = nc.declare_dram_parameter("out", [NQH, VD], F32, isOutput=True)

    NQC = NQH // 128  # 4 query chunks
    NKC = NK // 128   # 8 key chunks

    with tile.TileContext(nc) as tc:
        with (
            tc.tile_pool(name="consts", bufs=1) as consts,
            tc.tile_pool(name="io", bufs=1) as io,
            tc.tile_pool(name="chunks", bufs=4) as chunks,
            tc.tile_pool(name="vals", bufs=NKC) as vals,
            tc.tile_pool(name="work", bufs=3) as work,
            tc.tile_pool(name="jbank", bufs=3) as jbank,
            tc.tile_pool(name="sm", bufs=NKC) as sm,
            tc.tile_pool(name="ps", bufs=8, space="PSUM") as ps,
        ):
            # ---- constants ----
            id32 = consts.tile([128, 128], F32, tag="id32")
            make_identity(nc, id32[:])
            ones16 = consts.tile([128, 1], BF16, tag="ones16")
            nc.gpsimd.memset(ones16[:], 1.0)
            wq2_sb = consts.tile([QD, 128], F32, tag="wq2")
            wk2_sb = consts.tile([KD, 128], F32, tag="wk2")
            cw_sb = consts.tile([128, J], F32, tag="cw")
            sphq_sb = consts.tile([128, 1], F32, tag="sphq")
            sphk_sb = consts.tile([128, 1], F32, tag="sphk")
            biasq_sb = consts.tile([128, 1], F32, tag="biasq")
            biask_sb = consts.tile([128, 1], F32, tag="biask")
            nc.sync.dma_start(wq2_sb[:], wq2[:])
            nc.sync.dma_start(wk2_sb[:], wk2[:])
            nc.sync.dma_start(cw_sb[:], cw[:])
            nc.sync.dma_start(sphq_sb[:], sphq[:])
            nc.sync.dma_start(sphk_sb[:], sphk[:])
            nc.sync.dma_start(biasq_sb[:], biasq[:])
            nc.sync.dma_start(biask_sb[:], biask[:])

            # ---- inputs: q/k spread across queues, transpose to (d, n) ----
            qT = io.tile([QD, NQH], F32, tag="qT")
            kT = io.tile([KD, NK], F32, tag="kT")
            for i in range(NQC):
                qc = chunks.tile([128, QD], F32, tag="qc")
                nc.sync.dma_start(qc[:], q_sh[i * 128:(i + 1) * 128, :])
                p = ps.tile([128, 512], F32, tag="t512")
                nc.tensor.transpose(p[:, :128], qc[:], id32[:])
                nc.vector.tensor_copy(qT[:, i * 128:(i + 1) * 128], p[:, :128])
            for i in range(NKC):
                kc_t = chunks.tile([128, KD], F32, tag="kc")
                nc.scalar.dma_start(kc_t[:], k_sh[i * 128:(i + 1) * 128, :])
                p = ps.tile([128, 512], F32, tag="t512")
                nc.tensor.transpose(p[:, :128], kc_t[:], id32[:])
                nc.vector.tensor_copy(kT[:, i * 128:(i + 1) * 128], p[:, :128])
            # values: needed only at the tail; own queue, cast to bf16
            v16 = []
            for i in range(NKC):
                vc = chunks.tile([128, VD], F32, tag="vc")
                nc.gpsimd.dma_start(vc[:], v_sh[i * 128:(i + 1) * 128, :])
                vb = vals.tile([128, VD], BF16, tag="v16", name=f"v16_{i}")
                nc.vector.tensor_copy(vb[:], vc[:])
                v16.append(vb)

            # ---- projections: packed (2x64 h, n) = [W | W]^T @ xT ----
            qp2 = io.tile([128, NQH], F32, tag="qp2")
            kp2 = io.tile([128, NK], F32, tag="kp2")
            p = ps.tile([128, 512], F32, tag="t512")
            nc.tensor.matmul(p[:], wq2_sb[:], qT[:], start=True, stop=True)
            nc.scalar.copy(qp2[:], p[:])
            for c in range(2):
                p = ps.tile([128, 512], F32, tag="t512")
                nc.tensor.matmul(p[:], wk2_sb[:], kT[:, c * 512:(c + 1) * 512],
                                 start=True, stop=True)
                nc.scalar.copy(kp2[:, c * 512:(c + 1) * 512], p[:])

            # ---- per-j banks + transposed score accumulation over j ----
            # Q rows [sin | cos] scaled by c_j*w_v (bf16); K rows [cos | sin].
            psT = [ps.tile([128, 512], F32, tag="t512", name=f"psT_{kt}")
                   for kt in range(NKC)]

            for j in range(J):
                ks = jbank.tile([128, NK], BF16, tag="ks", name=f"ks_{j}")
                sq_f = work.tile([128, NQH], F32, tag="sqf", name=f"sqf_{j}")
                sq = jbank.tile([128, NQH], BF16, tag="sq", name=f"sq_{j}")
                if j == 0:  # |w0 x + pi/2| < 2.9: direct activation
                    nc.scalar.activation(ks[:], kp2[:], A.Sin,
                                         bias=biask_sb[:], scale=W0)
                    nc.scalar.activation(sq_f[:], qp2[:], A.Sin,
                                         bias=biasq_sb[:], scale=W0)
                else:
                    s1 = float(FOURIER_W[j] / TWO_PI)
                    zk = work.tile([128, NK], F32, tag="zk", name=f"zk_{j}")
                    rk = work.tile([128, NK], F32, tag="rk", name=f"rk_{j}")
                    gk = work.tile([128, NK], F32, tag="gk", name=f"gk_{j}")
                    nc.vector.tensor_scalar(zk[:], kp2[:], s1, sphk_sb[:],
                                            AluOpType.mult, AluOpType.add)
                    nc.vector.tensor_scalar(rk[:], zk[:], MAGIC, MAGIC,
                                            AluOpType.add, AluOpType.subtract)
                    nc.vector.tensor_sub(gk[:], rk[:], zk[:])
                    nc.scalar.activation(ks[:], gk[:], A.Sin, scale=-TWO_PI)
                    zq = work.tile([128, NQH], F32, tag="zq", name=f"zq_{j}")
                    rq = work.tile([128, NQH], F32, tag="rq", name=f"rq_{j}")
                    gq = work.tile([128, NQH], F32, tag="gq", name=f"gq_{j}")
                    nc.vector.tensor_scalar(zq[:], qp2[:], s1, sphq_sb[:],
                                            AluOpType.mult, AluOpType.add)
                    nc.vector.tensor_scalar(rq[:], zq[:], MAGIC, MAGIC,
                                            AluOpType.add, AluOpType.subtract)
                    nc.vector.tensor_sub(gq[:], rq[:], zq[:])
                    nc.scalar.activation(sq_f[:], gq[:], A.Sin, scale=-TWO_PI)
                # c_j*w_v scaling + bf16 cast on ScalarE (Copy with AP scale)
                nc.scalar.mul(sq[:], sq_f[:], cw_sb[:, j:j + 1])
                for kt in range(NKC):
                    nc.tensor.matmul(psT[kt][:],
                                     ks[:, kt * 128:(kt + 1) * 128], sq[:],
                                     start=(j == 0), stop=(j == J - 1))

            # ---- exp (k-major) + denominators via ones-matmul ----
            expT = []
            for kt in range(NKC):
                et = sm.tile([128, 512], BF16, tag="expT", name=f"expT_{kt}")
                nc.scalar.activation(et[:], psT[kt][:], A.Exp)
                expT.append(et)
            psum_sums = ps.tile([1, 512], F32, tag="t512", name="psum_sums")
            for kt in range(NKC):
                nc.tensor.matmul(psum_sums[:], ones16[:], expT[kt][:],
                                 start=(kt == 0), stop=(kt == NKC - 1))
            sums_sb = sm.tile([1, 512], F32, tag="sums_sb")
            nc.scalar.copy(sums_sb[:], psum_sums[:])

            # ---- outT (v, q) = sum_kt values[kt] (as lhsT) @ expT[kt] ----
            ps_outT = ps.tile([128, 512], F32, tag="t512", name="ps_outT")
            for kt in range(NKC):
                nc.tensor.matmul(ps_outT[:], v16[kt][:], expT[kt][:],
                                 start=(kt == 0), stop=(kt == NKC - 1))
            outT_sb = sm.tile([128, 512], F32, tag="outT_sb")
            nc.vector.tensor_copy(outT_sb[:], ps_outT[:])

            # ---- transpose back to (q, v), normalize, store ----
            for qt in range(NQC):
                pcol = ps.tile([128, 512], F32, tag="t512", name=f"pcol_{qt}")
                # (1,128) row -> (128,1) column via 1-deep matmul against [[1.0]]
                nc.tensor.matmul(pcol[:128, :1],
                                 sums_sb[:1, qt * 128:(qt + 1) * 128],
                                 id32[:1, :1], start=True, stop=True)
                rcol = sm.tile([128, 1], F32, tag="rcol", name=f"rcol_{qt}")
                nc.vector.reciprocal(rcol[:], pcol[:128, :1])
                po = ps.tile([128, 512], F32, tag="t512", name=f"po_{qt}")
                nc.tensor.transpose(po[:, :128],
                                    outT_sb[:, qt * 128:(qt + 1) * 128], id32[:])
                o_sb = sm.tile([128, VD], F32, tag="osb", name=f"osb_{qt}")
                nc.vector.tensor_scalar_mul(o_sb[:], po[:, :128], rcol[:])
                nc.sync.dma_start(out[qt * 128:(qt + 1) * 128, :], o_sb[:])

    nc.finalize()
    return nc


def _get_nc():
    if "nc" not in _CACHED:
        _CACHED["nc"] = _build()
    return _CACHED["nc"]


def _make_consts(W_q, W_k, w_v):
    wq2 = np.concatenate([W_q, W_q], axis=1).astype(np.float32)
    wk2 = np.concatenate([W_k, W_k], axis=1).astype(np.float32)
    cw = np.zeros((128, J), np.float32)
    for j in range(J):
        cwj = (FOURIER_C[j] * w_v).astype(np.float32)
        cw[:64, j] = cwj
        cw[64:, j] = cwj
    # wrap-phase consts (turns): Q packing [sin | cos], K packing [cos | sin]
    sphq = np.full((128, 1), 32.0, np.float32)
    sphq[64:] = 32.25
    sphk = np.full((128, 1), 32.25, np.float32)
    sphk[64:] = 32.0
    # direct-path (j=0) activation bias in radians
    biasq = np.zeros((128, 1), np.float32)
    biasq[64:] = HALF_PI
    biask = np.full((128, 1), HALF_PI, np.float32)
    biask[64:] = 0.0
    return wq2, wk2, cw, sphq, sphk, biasq, biask


def kernel(queries, keys, values, W_q, W_k, w_v, _trace=False, _trace_kwargs=None):
    from concourse.bass_utils import run_bass_kernel_spmd

    nc = _get_nc()
    wq2, wk2, cw, sphq, sphk, biasq, biask = _make_consts(
        np.asarray(W_q), np.asarray(W_k), np.asarray(w_v))
    queries = np.ascontiguousarray(queries, np.float32)
    keys = np.ascontiguousarray(keys, np.float32)
    values = np.ascontiguousarray(values, np.float32)

    in_maps = []
    for c in range(NCORES):
        b, qh = c // 2, c % 2
        in_maps.append({
            "q_sh": np.ascontiguousarray(queries[b, qh * NQH:(qh + 1) * NQH, :]),
            "k_sh": keys[b],
            "v_sh": values[b],
            "wq2": wq2, "wk2": wk2, "cw": cw, "sphq": sphq, "sphk": sphk,
            "biasq": biasq, "biask": biask,
        })

    kwargs = {}
    if _trace:
        kwargs["trace"] = True
        kwargs.update(_trace_kwargs or {})
    res = run_bass_kernel_spmd(nc, in_maps, core_ids=list(range(NCORES)), **kwargs)

    out = np.empty((BS, NQ, VD), np.float32)
    for c in range(NCORES):
        b, qh = c // 2, c % 2
        out[b, qh * NQH:(qh + 1) * NQH, :] = res.results[c]["out"]
    if _trace:
        return out, res
    return out
